# revision 53
# baseline (speedup 1.0000x reference)
"""Fused Mllama-style text self-attention on one TRN2 chip (8 NeuronCores).

Sharding: tensor-parallel over heads (4 q heads / 1 kv head per core) for the
QKV projections + RoPE + attention; per-head AllToAlls reshard the attention
outputs to token-parallel, so each core computes a 512-token slice of the final
output projection against the full wo. Host code transposes operands so every
matmul contraction lands on the partition dimension, and re-assembles the
token-sliced outputs.

Perf structure:
- Q/K projections run in fp8(e4m3) DoubleRow mode (2 k-tiles per pass). Inputs
  are pre-scaled by 64 on the host; the combined 4096x factor on q/k is folded
  into the exp() input scale of the softmax, so no descale ops are needed.
- Softmax denominators accumulate on the PE (M=1 ones-matmuls) instead of DVE
  adds; 1/den uses the fast DVE reciprocal on a single [1,QB] row, and is
  broadcast across partitions with a K=1 matmul.
- Phase 3's first wo column-block is prefetched into SBUF during phase 2 on
  the sync queue, and each head's A2A output is pulled into SBUF right after
  its collective (gpsimd queue), so the output projection starts immediately
  after attention and the last head's A2A hides under the first 24
  accumulation steps.
- Phase-3 PSUM groups are split in half (4 banks x 2 bufs) so drains overlap
  the next accumulation group.

kernel(**inputs) takes the FULL (unsharded) inputs and returns the FULL output.
"""

import math

import numpy as np
import ml_dtypes

import concourse.bacc as bacc
import concourse.bass as bass
import concourse.mybir as mybir
import concourse.tile as tile
from concourse.bass_utils import run_bass_kernel_spmd

F32 = mybir.dt.float32
BF16 = mybir.dt.bfloat16
F8 = mybir.dt.float8e4
AF = mybir.ActivationFunctionType
ALU = mybir.AluOpType
DR = mybir.MatmulPerfMode.DoubleRow

NH, NKV, HD = 32, 8, 128
NEG = -1.0e9
N_CORES = 8
QS = 64.0  # host-side fp8 pre-scale for hidden/wq/wk


def build(T, S, H, compute_dtype="bf16", causal=True, n_cores=N_CORES):
    """Build the SPMD Bass program (same program for all cores).

    T: total tokens (B*S); S: seq len per batch; H: hidden size.
    """
    B = T // S
    TC = T // n_cores          # tokens per core in the output projection
    QHC = NH // n_cores        # local q heads (4)
    D = QHC * HD               # local q width (512)
    HT = H // 128              # contraction tiles over hidden
    QB = min(512, TC)          # attention query block width
    NQB = S // QB              # query blocks per batch
    KB = QB // 128             # 128-k-tiles per query block
    NKT = S // 128             # k tiles per batch
    NMB = H // 512             # output-projection column blocks
    NT = TC // 128             # output-projection row tiles
    CD = BF16 if compute_dtype == "bf16" else F32
    use_fp8 = compute_dtype == "bf16" and causal
    ISQ = 1.0 / math.sqrt(HD)
    # q,k carry a (QS*QS) factor each when fp8 is used; fold into exp scale
    ESC = ISQ / ((QS * QS) ** 2) if use_fp8 else ISQ
    TI = 512                   # tokens per QKV iteration
    NIT = T // TI
    nh2 = HT // 2

    nc = bacc.Bacc("TRN2", target_bir_lowering=False, debug=False,
                   enable_asserts=True, num_devices=n_cores)

    # hidden states arrive pre-tiled on the host: [128, NIT*2, nh2*TI] so
    # every SBUF tile loads as one fully-contiguous row per partition
    hsR = nc.declare_dram_parameter("hsR", [128, NIT * 2, nh2 * TI], CD,
                                    isOutput=False)
    if use_fp8:
        hs8R = nc.declare_dram_parameter("hs8R", [128, NIT * 2, nh2 * TI], F8,
                                         isOutput=False)
        wq8T = nc.declare_dram_parameter("wq8T", [128, HT * D], F8,
                                         isOutput=False)
        wk8T = nc.declare_dram_parameter("wk8T", [128, HT * HD], F8,
                                         isOutput=False)
    else:
        wqT = nc.declare_dram_parameter("wqT", [128, HT * D], CD,
                                        isOutput=False)
        wkT = nc.declare_dram_parameter("wkT", [128, HT * HD], CD,
                                        isOutput=False)
    wvT = nc.declare_dram_parameter("wvT", [128, HT * HD], CD, isOutput=False)
    # wo pre-tiled: [128, NH, NMB/2, 1024] - contiguous per partition per tile
    woT = nc.declare_dram_parameter("woT", [128, NH, NMB // 2, 1024], CD,
                                    isOutput=False)
    cosT = nc.declare_dram_parameter("cosT", [HD, S], F32, isOutput=False)
    sgnT = nc.declare_dram_parameter("sgnT", [HD, S], F32, isOutput=False)
    if causal:
        dmask = nc.declare_dram_parameter("dmask", [128, KB * QB], CD, isOutput=False)
    else:
        maskT = nc.declare_dram_parameter("maskT", [S, S], F32, isOutput=False)
    out_c = nc.declare_dram_parameter("out", [TC, H], F32, isOutput=True)

    with tile.TileContext(nc) as tc:
        with tc.tile_pool(name="persist", bufs=1) as per, \
             tc.tile_pool(name="dram", bufs=1, space="DRAM") as dram:
            # persistent SBUF tensors
            qt = per.tile([128, QHC * T], CD)      # rope'd Q, head-major [d, t]
            kt = per.tile([128, T], CD)            # rope'd K [d, t]
            vt = per.tile([128, T], CD)            # V tiles [t(128), d] at col k*128
            cs = per.tile([128, S], F32)
            sg = per.tile([128, S], F32)
            ones = per.tile([128, 128], CD)
            nc.gpsimd.memset(ones[:], 1.0)
            if causal:
                dm = per.tile([128, KB * QB], CD)

            # per-head A2A bounce buffers
            a2a_in = [dram.tile([n_cores, 128, TC], CD, name=f"a2a_in{i}")
                      for i in range(QHC)]
            a2a_out = [dram.tile([n_cores, 128, TC], CD, name=f"a2a_out{i}")
                       for i in range(QHC)]

            # ---------------- Phase 1: QKV projections + RoPE ----------------
            with tc.tile_pool(name="wq", bufs=1) as wqp, \
                 tc.tile_pool(name="hst", bufs=3) as hstp, \
                 tc.tile_pool(name="hst8", bufs=5) as hst8p, \
                 tc.tile_pool(name="qkps", bufs=2, space="PSUM") as qkps, \
                 tc.tile_pool(name="vps", bufs=4, space="PSUM") as vps, \
                 tc.tile_pool(name="epi", bufs=3) as epi:
                WD = F8 if use_fp8 else CD
                wq_sb = wqp.tile([128, HT * D], WD)
                wk_sb = wqp.tile([128, HT * HD], WD)
                wv_sb = wqp.tile([128, HT * HD], CD)
                wq_src = wq8T if use_fp8 else wqT
                wk_src = wk8T if use_fp8 else wkT
                # projection weights (host pre-tiled, contiguous rows) split
                # across both hwdge queues; cos/sin/mask follow on the Act
                # ring (first needed ~60us in)
                hw = HT * D // 2
                nc.scalar.dma_start(wq_sb[:, 0:hw], wq_src[:, 0:hw])
                if use_fp8:
                    # land iteration-0's fp8 activations before the second
                    # weight half: the first matmuls gate on (wq h0 + these)
                    hs80 = hst8p.tile([128, nh2 * TI], F8, tag="hsp8",
                                      name="hsp8_0_0")
                    nc.sync.dma_start(hs80[:], hs8R[:, 0, :])
                    hs81 = hst8p.tile([128, nh2 * TI], F8, tag="hsp8",
                                      name="hsp8_0_1")
                    nc.sync.dma_start(hs81[:], hs8R[:, 1, :])
                nc.sync.dma_start(wq_sb[:, hw:], wq_src[:, hw:])
                nc.sync.dma_start(wk_sb[:], wk_src[:])
                nc.scalar.dma_start(wv_sb[:], wvT[:])
                nc.scalar.dma_start(cs[:], cosT[:])
                nc.scalar.dma_start(sg[:], sgnT[:])
                if causal:
                    nc.scalar.dma_start(dm[:], dmask[:])
                # [p, ht2, j, m] views for DoubleRow weight slices
                wq4 = wq_sb[:].rearrange("p (h j m) -> p h j m", h=nh2, j=2)
                wk4 = wk_sb[:].rearrange("p (h j m) -> p h j m", h=nh2, j=2)

                def rope(pA, pB, dst_ap, sc):
                    # dst = ab*cos + rotate_half(ab)*sin with ab = pA + pB.
                    # The half-rotation crosses partitions, which compute
                    # engines cannot do SBUF->SBUF, so shift via on-chip DMA.
                    ab = epi.tile([128, TI], F32, tag="ab", name="ab")
                    nc.scalar.activation(ab[:], pA[:], AF.Copy)
                    nc.vector.tensor_add(ab[:], ab[:], pB[:])
                    sh = epi.tile([128, TI], F32, tag="sh", name="sh")
                    # on the gpsimd queue: the sync ring is busy with hs tiles
                    # and a delayed shift stalls the PSUM ring for the PE
                    nc.gpsimd.dma_start(sh[0:64, :], ab[64:128, :])
                    nc.gpsimd.dma_start(sh[64:128, :], ab[0:64, :])
                    x1 = epi.tile([128, TI], F32, tag="x1", name="x1")
                    nc.vector.tensor_mul(x1[:], ab[:], cs[:, sc:sc + TI])
                    nc.vector.tensor_mul(sh[:], sh[:], sg[:, sc:sc + TI])
                    nc.vector.tensor_add(dst_ap, x1[:], sh[:])

                for it in range(NIT):
                    t0 = it * TI
                    sc = t0 % S  # column into cos/sgn tables
                    hs_ts = []
                    qk_rhs = []  # [128, 2, TI] DoubleRow rhs per ht-pair
                    if use_fp8:
                        for half in range(2):
                            if it == 0:
                                hsp8 = (hs80, hs81)[half]
                            else:
                                hsp8 = hst8p.tile([128, nh2 * TI], F8,
                                                  tag="hsp8",
                                                  name=f"hsp8_{it}_{half}")
                                nc.sync.dma_start(hsp8[:],
                                                  hs8R[:, it * 2 + half, :])
                            h3 = hsp8[:].rearrange("p (a t) -> p a t", a=nh2)
                            for j in range(nh2 // 2):
                                qk_rhs.append(h3[:, 2 * j:2 * j + 2, :])
                    for half in range(2):
                        hsp = hstp.tile([128, nh2 * TI], CD, tag="hsp",
                                        name=f"hsp_{it}_{half}")
                        nc.sync.dma_start(hsp[:], hsR[:, it * 2 + half, :])
                        for j in range(nh2):
                            hs_ts.append(hsp[:, j * TI:(j + 1) * TI])
                    # q heads + k: accumulate over ht alternating two PSUM
                    # banks (avoids same-bank drain serialization), then
                    # combine A+B in the epilogue.
                    for g in range(QHC + 1):  # 4 q heads then k
                        pA = qkps.tile([128, TI], F32, tag="pA",
                                       name=f"pA_{it}_{g}")
                        pB = qkps.tile([128, TI], F32, tag="pB",
                                       name=f"pB_{it}_{g}")
                        if use_fp8:
                            for h2 in range(nh2):
                                if g < QHC:
                                    w_ap = wq4[:, h2, :, g * 128:(g + 1) * 128]
                                else:
                                    w_ap = wk4[:, h2, :, :]
                                dst = pA if h2 % 2 == 0 else pB
                                nc.tensor.matmul(dst[:], w_ap, qk_rhs[h2],
                                                 start=(h2 < 2),
                                                 stop=(h2 >= nh2 - 2),
                                                 perf_mode=DR)
                        else:
                            for ht in range(HT):
                                if g < QHC:
                                    w_ap = wq_sb[:, ht * D + g * 128:
                                                 ht * D + (g + 1) * 128]
                                else:
                                    w_ap = wk_sb[:, ht * HD:(ht + 1) * HD]
                                dst = pA if ht % 2 == 0 else pB
                                nc.tensor.matmul(dst[:], w_ap, hs_ts[ht],
                                                 start=(ht < 2),
                                                 stop=(ht >= HT - 2))
                        if g < QHC:
                            rope(pA, pB, qt[:, g * T + t0: g * T + t0 + TI], sc)
                        else:
                            rope(pA, pB, kt[:, t0:t0 + TI], sc)
                    # v: [t,128] x wv groups. ht-outer so the 4 tsub
                    # accumulations interleave across 4 PSUM banks - a
                    # single-bank chain of N=128 matmuls serializes on the
                    # ~173ns PE<->PSUM pipeline latency
                    vp4 = [vps.tile([128, 128], F32, tag="vp",
                                    name=f"vp_{it}_{tsub}")
                           for tsub in range(TI // 128)]
                    for ht in range(HT):
                        for tsub in range(TI // 128):
                            nc.tensor.matmul(
                                vp4[tsub][:],
                                hs_ts[ht][:, tsub * 128:(tsub + 1) * 128],
                                wv_sb[:, ht * HD:(ht + 1) * HD],
                                start=(ht == 0), stop=(ht == HT - 1))
                    for tsub in range(TI // 128):
                        nc.scalar.activation(
                            vt[:, t0 + tsub * 128: t0 + (tsub + 1) * 128],
                            vp4[tsub][:], AF.Copy)

            tc.no_sync_barrier()
            d_order = [i * QHC + hl for hl in range(QHC) for i in range(n_cores)]
            with tc.tile_pool(name="otl", bufs=1) as otlp, \
                 tc.tile_pool(name="wot", bufs=34) as wotp:
                ot_loc = otlp.tile([128, NH * TC], CD)
                # prefetch wo column-block 0 during attention. On the gpsimd
                # queue: a sync-queue prefetch makes phase-2's first matmul
                # transitively wait on it (queue-counting semaphores)
                wot_tiles = {}
                for g in d_order:
                    wt = wotp.tile([128, 1024], CD, tag="wot",
                                   name=f"wot_0_{g}")
                    wot_tiles[(0, g)] = wt
                    nc.gpsimd.dma_start(wt[:], woT[:, g, 0, :])

                # ---------------- Phase 2: attention ----------------
                # ST pairs: two k-tiles share one [128, 2*QB] PSUM tile (two
                # banks), one exp per pair; causal masking multiplies the exp
                # output by a 0/1 pattern (cheap bf16 4x DVE); denominator rows
                # accumulate on PE via M=1 matmuls into PSUM; 1/den via the
                # fast DVE reciprocal on one row, broadcast with a K=1 matmul.
                with tc.tile_pool(name="stps", bufs=2, space="PSUM") as stps, \
                     tc.tile_pool(name="otps", bufs=2, space="PSUM") as otps, \
                     tc.tile_pool(name="dbps", bufs=2, space="PSUM") as dbps, \
                     tc.tile_pool(name="att", bufs=4) as att, \
                     tc.tile_pool(name="ep2", bufs=2) as ep2, \
                     tc.tile_pool(name="attm", bufs=3) as attm:
                    for hl in range(QHC):
                        for b in range(B):
                            for qb in range(NQB):
                                q0 = b * S + qb * QB          # global q col
                                n_k = (qb + 1) * KB if causal else NKT
                                otp = otps.tile([128, QB], F32, tag="ot",
                                                name=f"ot_{hl}_{b}_{qb}")
                                db = dbps.tile([128, QB], F32, tag="db",
                                               name=f"db_{hl}_{b}_{qb}")
                                acc = ep2.tile([128, QB], CD, tag="acc",
                                               name=f"acc_{hl}_{b}_{qb}")
                                for kp in range(n_k // 2):
                                    stp = stps.tile([128, 2 * QB], F32,
                                                    tag="st",
                                                    name=f"st_{hl}_{b}_{qb}_{kp}")
                                    for half in range(2):
                                        kti = 2 * kp + half
                                        kg = b * NKT + kti
                                        nc.tensor.matmul(
                                            stp[:, half * QB:(half + 1) * QB],
                                            kt[:, kg * 128:(kg + 1) * 128],
                                            qt[:, hl * T + q0:
                                               hl * T + q0 + QB],
                                            start=True, stop=True)
                                    pt = att.tile([128, 2 * QB], CD, tag="pt",
                                                  name=f"pt_{hl}_{b}_{qb}_{kp}")
                                    d0 = 2 * kp - qb * KB  # diag pattern index
                                    if causal and 2 * kp + 1 >= qb * KB:
                                        pr = att.tile([128, 2 * QB], CD,
                                                      tag="pr", name="pr")
                                        nc.scalar.activation(pr[:], stp[:],
                                                             AF.Exp, scale=ESC)
                                        nc.vector.tensor_mul(
                                            pt[:], pr[:],
                                            dm[:, d0 * QB:(d0 + 2) * QB])
                                    elif not causal:
                                        mt = attm.tile([128, 2 * QB], F32,
                                                       tag="mt", name="mt")
                                        for half in range(2):
                                            kti = 2 * kp + half
                                            nc.sync.dma_start(
                                                mt[:, half * QB:(half + 1) * QB],
                                                maskT[kti * 128:(kti + 1) * 128,
                                                      qb * QB:(qb + 1) * QB])
                                        tmp = att.tile([128, 2 * QB], F32,
                                                       tag="tmp", name="tmp")
                                        nc.vector.tensor_add(tmp[:], stp[:],
                                                             mt[:])
                                        nc.scalar.activation(pt[:], tmp[:],
                                                             AF.Exp, scale=ESC)
                                    else:
                                        nc.scalar.activation(pt[:], stp[:],
                                                             AF.Exp, scale=ESC)
                                    for half in range(2):
                                        kti = 2 * kp + half
                                        kg = b * NKT + kti
                                        nc.tensor.matmul(
                                            otp[:],
                                            vt[:, kg * 128:(kg + 1) * 128],
                                            pt[:, half * QB:(half + 1) * QB],
                                            start=(kti == 0),
                                            stop=(kti == n_k - 1))
                                    # denominator: bf16 DVE accumulation of
                                    # the per-tile exp sums; one PE partition-
                                    # reduce per block below
                                    if kp == 0:
                                        nc.vector.tensor_add(
                                            acc[:], pt[:, 0:QB],
                                            pt[:, QB:2 * QB])
                                    else:
                                        nc.vector.tensor_add(
                                            acc[:], acc[:], pt[:, 0:QB])
                                        nc.vector.tensor_add(
                                            acc[:], acc[:], pt[:, QB:2 * QB])
                                nc.tensor.matmul(db[0:1, :], ones[:, 0:1],
                                                 acc[:], start=True, stop=True)
                                rec1 = ep2.tile([1, QB], F32, tag="rec1",
                                                name="rec1")
                                nc.vector.reciprocal_approx_fast(rec1[:],
                                                                 db[0:1, :])
                                recb = ep2.tile([1, QB], CD, tag="recb",
                                                name="recb")
                                nc.vector.tensor_copy(recb[:], rec1[:])
                                nc.tensor.matmul(db[:], ones[0:1, :], recb[:],
                                                 start=True, stop=True)
                                dbs = ep2.tile([128, QB], CD, tag="dbs",
                                               name="dbs")
                                nc.scalar.activation(dbs[:], db[:], AF.Copy)
                                ot_sb = ep2.tile([128, QB], CD, tag="otsb",
                                                 name="otsb")
                                nc.vector.tensor_mul(ot_sb[:], otp[:], dbs[:])
                                # scatter into this head's A2A input buffer
                                nj = max(1, QB // TC)
                                j0 = q0 // TC
                                c0 = q0 % TC
                                if nj == 1:
                                    nc.sync.dma_start(
                                        a2a_in[hl][j0, :, c0:c0 + QB], ot_sb[:])
                                else:
                                    nc.sync.dma_start(
                                        a2a_in[hl][j0:j0 + nj, :, :],
                                        ot_sb.rearrange("p (j c) -> j p c",
                                                        j=nj))
                        nc.gpsimd.collective_compute(
                            "AllToAll", ALU.bypass,
                            replica_groups=[list(range(n_cores))],
                            ins=[a2a_in[hl][:]],
                            outs=[a2a_out[hl][:]])
                        # pull this head's reshard result into SBUF right away
                        # (gpsimd queue - keeps the sync queue free)
                        for i in range(n_cores):
                            g = i * QHC + hl
                            nc.gpsimd.dma_start(
                                ot_loc[:, g * TC:(g + 1) * TC],
                                a2a_out[hl][i, :, :])

                tc.no_sync_barrier()
                # ---------------- Phase 3: output projection ----------------
                # Each 1024-col wo block runs as two 512-col PSUM half-groups
                # (4 banks x 2 bufs) so drains overlap the next accumulation.
                with tc.tile_pool(name="ops", bufs=2, space="PSUM") as ops, \
                     tc.tile_pool(name="oout", bufs=4) as ooutp:

                    def prefetch_wo(mp):
                        # stream the next wo block on BOTH hwdge queues
                        # (sync + act) - one ring can't keep up with the
                        # PE's weight consumption rate
                        for gi2, g in enumerate(d_order):
                            wt = wotp.tile([128, 1024], CD, tag="wot",
                                           name=f"wot_{mp}_{g}")
                            wot_tiles[(mp, g)] = wt
                            eng = nc.sync if gi2 % 2 == 0 else nc.scalar
                            eng.dma_start(wt[:], woT[:, g, mp, :])

                    def mm(pos, sub, gi):
                        g = d_order[gi]
                        wt = wot_tiles[(mp, g)]
                        for tt in range(NT):
                            nc.tensor.matmul(
                                pos[tt],
                                ot_loc[:, g * TC + tt * 128:
                                       g * TC + (tt + 1) * 128],
                                wt[:, sub * 512:(sub + 1) * 512],
                                start=(gi == 0), stop=(gi == NH - 1))

                    def drain(pos, mp, sub):
                        # alternate PSUM drains between Act and the (idle in
                        # phase 3) DVE so pos banks free for the next group
                        # twice as fast
                        for tt in range(NT):
                            ob = ooutp.tile([128, 512], F32, tag="ob",
                                            name="ob")
                            if tt % 2 == 0:
                                nc.scalar.activation(ob[:], pos[tt], AF.Copy)
                            else:
                                nc.vector.tensor_copy(ob[:], pos[tt])
                            nc.sync.dma_start(
                                out_c[tt * 128:(tt + 1) * 128,
                                      (2 * mp + sub) * 512:
                                      (2 * mp + sub + 1) * 512],
                                ob[:])

                    for mp in range(NMB // 2):
                        if mp == 0:
                            # interleave both PSUM half-groups in gi-chunks:
                            # the first 24 g's (heads 0-2) give the PE ~190
                            # matmuls of runway that hides head 3's A2A
                            pos2 = [[ops.tile([128, 512], F32,
                                              tag=f"po{tt}",
                                              name=f"po_0_{sub}_{tt}")
                                     for tt in range(NT)]
                                    for sub in range(2)]
                            for sub in range(2):
                                for gi in range(0, 3 * NH // 4):
                                    mm(pos2[sub], sub, gi)
                            prefetch_wo(1)
                            for sub in range(2):
                                for gi in range(3 * NH // 4, NH):
                                    mm(pos2[sub], sub, gi)
                            for sub in range(2):
                                drain(pos2[sub], 0, sub)
                        else:
                            for sub in range(2):
                                pos = [ops.tile([128, 512], F32,
                                                tag=f"po{tt}",
                                                name=f"po_{mp}_{sub}_{tt}")
                                       for tt in range(NT)]
                                for gi in range(NH):
                                    mm(pos, sub, gi)
                                if sub == 0 and mp + 1 < NMB // 2:
                                    prefetch_wo(mp + 1)
                                drain(pos, mp, sub)

    nc.compile()
    return nc


def _np16(x):
    return np.asarray(x, dtype=ml_dtypes.bfloat16)


def _np8(x):
    return np.asarray(np.asarray(x, np.float32) * QS, dtype=ml_dtypes.float8_e4m3)


def prep_inputs(hidden_states, attention_mask, cos, sin, wq, wk, wv, wo,
                compute_dtype="bf16", n_cores=N_CORES):
    """Host-side sharding + transposes. Returns (in_maps, causal, dims)."""
    B, S, H = hidden_states.shape
    T = B * S
    D = NH * HD // n_cores
    KD = NKV * HD // n_cores
    cd = (lambda x: _np16(x)) if compute_dtype == "bf16" else \
         (lambda x: np.ascontiguousarray(x, dtype=np.float32))

    m = np.asarray(attention_mask, np.float32)[0, 0]
    expected = np.where(np.tril(np.ones((S, S), bool)), 0.0, NEG).astype(np.float32)
    causal = bool(np.array_equal(m, expected))
    use_fp8 = compute_dtype == "bf16" and causal

    hs2 = np.asarray(hidden_states, np.float32).reshape(T, H)
    # pre-tile hidden states: [128, NIT*2, nh2*TI] with one contiguous row
    # per partition per SBUF tile (TI=512 tokens, nh2=16 h-tiles per half)
    TI, nh2 = 512, H // 256
    NIT = T // TI
    hs4 = hs2.T.reshape(2, nh2, 128, NIT, TI)
    hsR = np.ascontiguousarray(hs4.transpose(2, 3, 0, 1, 4)).reshape(
        128, NIT * 2, nh2 * TI)

    def wtile(w, cols):
        # [cols*HT_rows, ...] -> [128, HT*cols]: tile ht on free dim
        wT = np.asarray(w, np.float32).T      # [H, cols]
        ht = H // 128
        return np.ascontiguousarray(
            wT.reshape(ht, 128, cols).transpose(1, 0, 2)).reshape(128, ht * cols)

    # wo: [128, NH, NMB/2, 1024] with p = row within 128-block g
    wo4 = np.asarray(wo, np.float32).T.reshape(NH, 128, H // 1024, 1024)
    woR = np.ascontiguousarray(wo4.transpose(1, 0, 2, 3))
    woT = cd(woR)
    cosT = np.ascontiguousarray(np.asarray(cos, np.float32)[0].T)
    sinT = np.ascontiguousarray(np.asarray(sin, np.float32)[0].T)
    sgnT = np.concatenate([-sinT[0:HD // 2], sinT[HD // 2:]], axis=0)
    sgnT = np.ascontiguousarray(sgnT)
    hsRc = cd(hsR)
    if use_fp8:
        hs8R = _np8(hsR)

    TC = T // n_cores
    QB = min(512, TC)
    KB = QB // 128
    in_maps = []
    for c in range(n_cores):
        im = {
            "hsR": hsRc,
            "wvT": cd(wtile(np.asarray(wv, np.float32)[c * KD:(c + 1) * KD],
                            KD)),
            "woT": woT,
            "cosT": cosT,
            "sgnT": sgnT,
        }
        wqs = np.asarray(wq, np.float32)[c * D:(c + 1) * D]
        wks = np.asarray(wk, np.float32)[c * KD:(c + 1) * KD]
        if use_fp8:
            im["hs8R"] = hs8R
            im["wq8T"] = np.asarray(wtile(wqs, D) * QS,
                                    dtype=ml_dtypes.float8_e4m3)
            im["wk8T"] = np.asarray(wtile(wks, KD) * QS,
                                    dtype=ml_dtypes.float8_e4m3)
        else:
            im["wqT"] = cd(wtile(wqs, D))
            im["wkT"] = cd(wtile(wks, KD))
        if causal:
            pk = np.arange(128)[:, None]
            pq = np.arange(QB)[None, :]
            dmask = np.concatenate(
                [np.where(pk + j * 128 <= pq, 1.0, 0.0) for j in range(KB)],
                axis=1).astype(np.float32)
            im["dmask"] = cd(dmask)
        else:
            im["maskT"] = np.ascontiguousarray(m.T)
        in_maps.append(im)
    return in_maps, causal, (T, S, H)


_BUILD_CACHE = {}


def kernel(hidden_states, attention_mask, cos, sin, wq, wk, wv, wo,
           compute_dtype="bf16", trace=False):
    B, S, H = hidden_states.shape
    T = B * S
    in_maps, causal, dims = prep_inputs(
        hidden_states, attention_mask, cos, sin, wq, wk, wv, wo,
        compute_dtype=compute_dtype)
    key = (T, S, H, compute_dtype, causal)
    if key not in _BUILD_CACHE:
        _BUILD_CACHE[key] = build(T, S, H, compute_dtype=compute_dtype,
                                  causal=causal)
    nc = _BUILD_CACHE[key]
    res = run_bass_kernel_spmd(nc, in_maps, core_ids=list(range(N_CORES)),
                               trace=trace)
    TC = T // N_CORES
    out = np.empty((T, H), np.float32)
    for c in range(N_CORES):
        out[c * TC:(c + 1) * TC] = res.results[c]["out"]
    if trace:
        kernel.last_exec_time_ns = res.exec_time_ns
        kernel.last_results = res
    return out.reshape(B, S, H)


# revision 54
# speedup vs baseline: 1.0501x; 1.0501x over previous
"""Fused Mllama-style text self-attention on one TRN2 chip (8 NeuronCores).

Sharding: tensor-parallel over heads (4 q heads / 1 kv head per core) for the
QKV projections + RoPE + attention; per-head AllToAlls reshard the attention
outputs to token-parallel, so each core computes a 512-token slice of the final
output projection against the full wo. Host code transposes operands so every
matmul contraction lands on the partition dimension, and re-assembles the
token-sliced outputs.

Perf structure:
- Q/K projections run in fp8(e4m3) DoubleRow mode (2 k-tiles per pass). Inputs
  are pre-scaled by 64 on the host; the combined 4096x factor on q/k is folded
  into the exp() input scale of the softmax, so no descale ops are needed.
- Softmax denominators accumulate on the PE (M=1 ones-matmuls) instead of DVE
  adds; 1/den uses the fast DVE reciprocal on a single [1,QB] row, and is
  broadcast across partitions with a K=1 matmul.
- Phase 3's first wo column-block is prefetched into SBUF during phase 2 on
  the sync queue, and each head's A2A output is pulled into SBUF right after
  its collective (gpsimd queue), so the output projection starts immediately
  after attention and the last head's A2A hides under the first 24
  accumulation steps.
- Phase-3 PSUM groups are split in half (4 banks x 2 bufs) so drains overlap
  the next accumulation group.

kernel(**inputs) takes the FULL (unsharded) inputs and returns the FULL output.
"""

import math

import numpy as np
import ml_dtypes

import concourse.bacc as bacc
import concourse.bass as bass
import concourse.mybir as mybir
import concourse.tile as tile
from concourse.bass_utils import run_bass_kernel_spmd

F32 = mybir.dt.float32
BF16 = mybir.dt.bfloat16
F8 = mybir.dt.float8e4
AF = mybir.ActivationFunctionType
ALU = mybir.AluOpType
DR = mybir.MatmulPerfMode.DoubleRow

NH, NKV, HD = 32, 8, 128
NEG = -1.0e9
N_CORES = 8
QS = 64.0  # host-side fp8 pre-scale for hidden/wq/wk


def build(T, S, H, compute_dtype="bf16", causal=True, n_cores=N_CORES):
    """Build the SPMD Bass program (same program for all cores).

    T: total tokens (B*S); S: seq len per batch; H: hidden size.
    """
    B = T // S
    TC = T // n_cores          # tokens per core in the output projection
    QHC = NH // n_cores        # local q heads (4)
    D = QHC * HD               # local q width (512)
    HT = H // 128              # contraction tiles over hidden
    QB = min(512, TC)          # attention query block width
    NQB = S // QB              # query blocks per batch
    KB = QB // 128             # 128-k-tiles per query block
    NKT = S // 128             # k tiles per batch
    NMB = H // 512             # output-projection column blocks
    NT = TC // 128             # output-projection row tiles
    CD = BF16 if compute_dtype == "bf16" else F32
    use_fp8 = compute_dtype == "bf16" and causal
    ISQ = 1.0 / math.sqrt(HD)
    # q,k carry a (QS*QS) factor each when fp8 is used; fold into exp scale
    ESC = ISQ / ((QS * QS) ** 2) if use_fp8 else ISQ
    TI = 512                   # tokens per QKV iteration
    NIT = T // TI
    nh2 = HT // 2

    nc = bacc.Bacc("TRN2", target_bir_lowering=False, debug=False,
                   enable_asserts=True, num_devices=n_cores)

    # hidden states arrive pre-tiled on the host: [128, NIT*2, nh2*TI] so
    # every SBUF tile loads as one fully-contiguous row per partition
    hsR = nc.declare_dram_parameter("hsR", [128, NIT * 2, nh2 * TI], CD,
                                    isOutput=False)
    if use_fp8:
        hs8R = nc.declare_dram_parameter("hs8R", [128, NIT * 2, nh2 * TI], F8,
                                         isOutput=False)
        wq8T = nc.declare_dram_parameter("wq8T", [128, HT * D], F8,
                                         isOutput=False)
        wk8T = nc.declare_dram_parameter("wk8T", [128, HT * HD], F8,
                                         isOutput=False)
    else:
        wqT = nc.declare_dram_parameter("wqT", [128, HT * D], CD,
                                        isOutput=False)
        wkT = nc.declare_dram_parameter("wkT", [128, HT * HD], CD,
                                        isOutput=False)
    wvT = nc.declare_dram_parameter("wvT", [128, HT * HD], CD, isOutput=False)
    # wo pre-tiled: [128, NH, NMB/2, 1024] - contiguous per partition per tile
    woT = nc.declare_dram_parameter("woT", [128, NH, NMB // 2, 1024], CD,
                                    isOutput=False)
    cosT = nc.declare_dram_parameter("cosT", [HD, S], F32, isOutput=False)
    sgnT = nc.declare_dram_parameter("sgnT", [HD, S], F32, isOutput=False)
    if causal:
        dmask = nc.declare_dram_parameter("dmask", [128, KB * QB], CD, isOutput=False)
    else:
        maskT = nc.declare_dram_parameter("maskT", [S, S], F32, isOutput=False)
    out_c = nc.declare_dram_parameter("out", [TC, H], F32, isOutput=True)

    with tile.TileContext(nc) as tc:
        with tc.tile_pool(name="persist", bufs=1) as per, \
             tc.tile_pool(name="dram", bufs=1, space="DRAM") as dram:
            # persistent SBUF tensors
            qt = per.tile([128, QHC * T], CD)      # rope'd Q, head-major [d, t]
            kt = per.tile([128, T], CD)            # rope'd K [d, t]
            vt = per.tile([128, T], CD)            # V tiles [t(128), d] at col k*128
            cs = per.tile([128, S], F32)
            sg = per.tile([128, S], F32)
            ones = per.tile([128, 128], CD)
            nc.gpsimd.memset(ones[:], 1.0)
            if causal:
                dm = per.tile([128, KB * QB], CD)

            # per-head A2A bounce buffers
            a2a_in = [dram.tile([n_cores, 128, TC], CD, name=f"a2a_in{i}")
                      for i in range(QHC)]
            a2a_out = [dram.tile([n_cores, 128, TC], CD, name=f"a2a_out{i}")
                       for i in range(QHC)]

            # ---------------- Phase 1: QKV projections + RoPE ----------------
            with tc.tile_pool(name="wq", bufs=1) as wqp, \
                 tc.tile_pool(name="hst", bufs=3) as hstp, \
                 tc.tile_pool(name="hst8", bufs=5) as hst8p, \
                 tc.tile_pool(name="qkps", bufs=2, space="PSUM") as qkps, \
                 tc.tile_pool(name="vps", bufs=4, space="PSUM") as vps, \
                 tc.tile_pool(name="epi", bufs=3) as epi:
                WD = F8 if use_fp8 else CD
                wq_sb = wqp.tile([128, HT * D], WD)
                wk_sb = wqp.tile([128, HT * HD], WD)
                wv_sb = wqp.tile([128, HT * HD], CD)
                wq_src = wq8T if use_fp8 else wqT
                wk_src = wk8T if use_fp8 else wkT
                # projection weights (host pre-tiled, contiguous rows) split
                # across both hwdge queues; cos/sin/mask follow on the Act
                # ring (first needed ~60us in)
                hw = HT * D // 2
                nc.scalar.dma_start(wq_sb[:, 0:hw], wq_src[:, 0:hw])
                nc.sync.dma_start(wq_sb[:, hw:], wq_src[:, hw:])
                nc.sync.dma_start(wk_sb[:], wk_src[:])
                nc.scalar.dma_start(wv_sb[:], wvT[:])
                nc.scalar.dma_start(cs[:], cosT[:])
                nc.scalar.dma_start(sg[:], sgnT[:])
                if causal:
                    nc.scalar.dma_start(dm[:], dmask[:])
                # [p, ht2, j, m] views for DoubleRow weight slices
                wq4 = wq_sb[:].rearrange("p (h j m) -> p h j m", h=nh2, j=2)
                wk4 = wk_sb[:].rearrange("p (h j m) -> p h j m", h=nh2, j=2)

                def rope(pA, pB, dst_ap, sc):
                    # dst = ab*cos + rotate_half(ab)*sin with ab = pA + pB.
                    # The half-rotation crosses partitions, which compute
                    # engines cannot do SBUF->SBUF, so shift via on-chip DMA.
                    ab = epi.tile([128, TI], F32, tag="ab", name="ab")
                    nc.scalar.activation(ab[:], pA[:], AF.Copy)
                    nc.vector.tensor_add(ab[:], ab[:], pB[:])
                    sh = epi.tile([128, TI], F32, tag="sh", name="sh")
                    # on the gpsimd queue: the sync ring is busy with hs tiles
                    # and a delayed shift stalls the PSUM ring for the PE
                    nc.gpsimd.dma_start(sh[0:64, :], ab[64:128, :])
                    nc.gpsimd.dma_start(sh[64:128, :], ab[0:64, :])
                    x1 = epi.tile([128, TI], F32, tag="x1", name="x1")
                    nc.vector.tensor_mul(x1[:], ab[:], cs[:, sc:sc + TI])
                    nc.vector.tensor_mul(sh[:], sh[:], sg[:, sc:sc + TI])
                    nc.vector.tensor_add(dst_ap, x1[:], sh[:])

                for it in range(NIT):
                    t0 = it * TI
                    sc = t0 % S  # column into cos/sgn tables
                    hs_ts = []
                    qk_rhs = []  # [128, 2, TI] DoubleRow rhs per ht-pair
                    if use_fp8:
                        for half in range(2):
                            hsp8 = hst8p.tile([128, nh2 * TI], F8, tag="hsp8",
                                              name=f"hsp8_{it}_{half}")
                            nc.sync.dma_start(hsp8[:],
                                              hs8R[:, it * 2 + half, :])
                            h3 = hsp8[:].rearrange("p (a t) -> p a t", a=nh2)
                            for j in range(nh2 // 2):
                                qk_rhs.append(h3[:, 2 * j:2 * j + 2, :])
                    for half in range(2):
                        hsp = hstp.tile([128, nh2 * TI], CD, tag="hsp",
                                        name=f"hsp_{it}_{half}")
                        nc.sync.dma_start(hsp[:], hsR[:, it * 2 + half, :])
                        for j in range(nh2):
                            hs_ts.append(hsp[:, j * TI:(j + 1) * TI])
                    # q heads + k: accumulate over ht alternating two PSUM
                    # banks (avoids same-bank drain serialization), then
                    # combine A+B in the epilogue.
                    for g in range(QHC + 1):  # 4 q heads then k
                        pA = qkps.tile([128, TI], F32, tag="pA",
                                       name=f"pA_{it}_{g}")
                        pB = qkps.tile([128, TI], F32, tag="pB",
                                       name=f"pB_{it}_{g}")
                        if use_fp8:
                            for h2 in range(nh2):
                                if g < QHC:
                                    w_ap = wq4[:, h2, :, g * 128:(g + 1) * 128]
                                else:
                                    w_ap = wk4[:, h2, :, :]
                                dst = pA if h2 % 2 == 0 else pB
                                nc.tensor.matmul(dst[:], w_ap, qk_rhs[h2],
                                                 start=(h2 < 2),
                                                 stop=(h2 >= nh2 - 2),
                                                 perf_mode=DR)
                        else:
                            for ht in range(HT):
                                if g < QHC:
                                    w_ap = wq_sb[:, ht * D + g * 128:
                                                 ht * D + (g + 1) * 128]
                                else:
                                    w_ap = wk_sb[:, ht * HD:(ht + 1) * HD]
                                dst = pA if ht % 2 == 0 else pB
                                nc.tensor.matmul(dst[:], w_ap, hs_ts[ht],
                                                 start=(ht < 2),
                                                 stop=(ht >= HT - 2))
                        if g < QHC:
                            rope(pA, pB, qt[:, g * T + t0: g * T + t0 + TI], sc)
                        else:
                            rope(pA, pB, kt[:, t0:t0 + TI], sc)
                    # v: [t,128] x wv groups. ht-outer so the 4 tsub
                    # accumulations interleave across 4 PSUM banks - a
                    # single-bank chain of N=128 matmuls serializes on the
                    # ~173ns PE<->PSUM pipeline latency
                    vp4 = [vps.tile([128, 128], F32, tag="vp",
                                    name=f"vp_{it}_{tsub}")
                           for tsub in range(TI // 128)]
                    for ht in range(HT):
                        for tsub in range(TI // 128):
                            nc.tensor.matmul(
                                vp4[tsub][:],
                                hs_ts[ht][:, tsub * 128:(tsub + 1) * 128],
                                wv_sb[:, ht * HD:(ht + 1) * HD],
                                start=(ht == 0), stop=(ht == HT - 1))
                    for tsub in range(TI // 128):
                        nc.scalar.activation(
                            vt[:, t0 + tsub * 128: t0 + (tsub + 1) * 128],
                            vp4[tsub][:], AF.Copy)

            tc.no_sync_barrier()
            d_order = [i * QHC + hl for hl in range(QHC) for i in range(n_cores)]
            with tc.tile_pool(name="otl", bufs=1) as otlp, \
                 tc.tile_pool(name="wot", bufs=34) as wotp:
                ot_loc = otlp.tile([128, NH * TC], CD)
                # prefetch wo column-block 0 during attention. On the gpsimd
                # queue: a sync-queue prefetch makes phase-2's first matmul
                # transitively wait on it (queue-counting semaphores)
                wot_tiles = {}
                for g in d_order:
                    wt = wotp.tile([128, 1024], CD, tag="wot",
                                   name=f"wot_0_{g}")
                    wot_tiles[(0, g)] = wt
                    nc.gpsimd.dma_start(wt[:], woT[:, g, 0, :])

                # ---------------- Phase 2: attention ----------------
                # ST pairs: two k-tiles share one [128, 2*QB] PSUM tile (two
                # banks), one exp per pair; causal masking multiplies the exp
                # output by a 0/1 pattern (cheap bf16 4x DVE); denominator rows
                # accumulate on PE via M=1 matmuls into PSUM; 1/den via the
                # fast DVE reciprocal on one row, broadcast with a K=1 matmul.
                with tc.tile_pool(name="stps", bufs=2, space="PSUM") as stps, \
                     tc.tile_pool(name="otps", bufs=2, space="PSUM") as otps, \
                     tc.tile_pool(name="dbps", bufs=2, space="PSUM") as dbps, \
                     tc.tile_pool(name="att", bufs=4) as att, \
                     tc.tile_pool(name="ep2", bufs=2) as ep2, \
                     tc.tile_pool(name="attm", bufs=3) as attm:
                    for hl in range(QHC):
                        for b in range(B):
                            for qb in range(NQB):
                                q0 = b * S + qb * QB          # global q col
                                n_k = (qb + 1) * KB if causal else NKT
                                otp = otps.tile([128, QB], F32, tag="ot",
                                                name=f"ot_{hl}_{b}_{qb}")
                                db = dbps.tile([128, QB], F32, tag="db",
                                               name=f"db_{hl}_{b}_{qb}")
                                acc = ep2.tile([128, QB], CD, tag="acc",
                                               name=f"acc_{hl}_{b}_{qb}")
                                for kp in range(n_k // 2):
                                    stp = stps.tile([128, 2 * QB], F32,
                                                    tag="st",
                                                    name=f"st_{hl}_{b}_{qb}_{kp}")
                                    for half in range(2):
                                        kti = 2 * kp + half
                                        kg = b * NKT + kti
                                        nc.tensor.matmul(
                                            stp[:, half * QB:(half + 1) * QB],
                                            kt[:, kg * 128:(kg + 1) * 128],
                                            qt[:, hl * T + q0:
                                               hl * T + q0 + QB],
                                            start=True, stop=True)
                                    pt = att.tile([128, 2 * QB], CD, tag="pt",
                                                  name=f"pt_{hl}_{b}_{qb}_{kp}")
                                    d0 = 2 * kp - qb * KB  # diag pattern index
                                    if causal and 2 * kp + 1 >= qb * KB:
                                        pr = att.tile([128, 2 * QB], CD,
                                                      tag="pr", name="pr")
                                        nc.scalar.activation(pr[:], stp[:],
                                                             AF.Exp, scale=ESC)
                                        nc.vector.tensor_mul(
                                            pt[:], pr[:],
                                            dm[:, d0 * QB:(d0 + 2) * QB])
                                    elif not causal:
                                        mt = attm.tile([128, 2 * QB], F32,
                                                       tag="mt", name="mt")
                                        for half in range(2):
                                            kti = 2 * kp + half
                                            nc.sync.dma_start(
                                                mt[:, half * QB:(half + 1) * QB],
                                                maskT[kti * 128:(kti + 1) * 128,
                                                      qb * QB:(qb + 1) * QB])
                                        tmp = att.tile([128, 2 * QB], F32,
                                                       tag="tmp", name="tmp")
                                        nc.vector.tensor_add(tmp[:], stp[:],
                                                             mt[:])
                                        nc.scalar.activation(pt[:], tmp[:],
                                                             AF.Exp, scale=ESC)
                                    else:
                                        nc.scalar.activation(pt[:], stp[:],
                                                             AF.Exp, scale=ESC)
                                    for half in range(2):
                                        kti = 2 * kp + half
                                        kg = b * NKT + kti
                                        nc.tensor.matmul(
                                            otp[:],
                                            vt[:, kg * 128:(kg + 1) * 128],
                                            pt[:, half * QB:(half + 1) * QB],
                                            start=(kti == 0),
                                            stop=(kti == n_k - 1))
                                    # denominator: bf16 DVE accumulation of
                                    # the per-tile exp sums; one PE partition-
                                    # reduce per block below
                                    if kp == 0:
                                        nc.vector.tensor_add(
                                            acc[:], pt[:, 0:QB],
                                            pt[:, QB:2 * QB])
                                    else:
                                        nc.vector.tensor_add(
                                            acc[:], acc[:], pt[:, 0:QB])
                                        nc.vector.tensor_add(
                                            acc[:], acc[:], pt[:, QB:2 * QB])
                                nc.tensor.matmul(db[0:1, :], ones[:, 0:1],
                                                 acc[:], start=True, stop=True)
                                rec1 = ep2.tile([1, QB], F32, tag="rec1",
                                                name="rec1")
                                nc.vector.reciprocal_approx_fast(rec1[:],
                                                                 db[0:1, :])
                                recb = ep2.tile([1, QB], CD, tag="recb",
                                                name="recb")
                                nc.vector.tensor_copy(recb[:], rec1[:])
                                nc.tensor.matmul(db[:], ones[0:1, :], recb[:],
                                                 start=True, stop=True)
                                dbs = ep2.tile([128, QB], CD, tag="dbs",
                                               name="dbs")
                                nc.scalar.activation(dbs[:], db[:], AF.Copy)
                                ot_sb = ep2.tile([128, QB], CD, tag="otsb",
                                                 name="otsb")
                                nc.vector.tensor_mul(ot_sb[:], otp[:], dbs[:])
                                # scatter into this head's A2A input buffer
                                nj = max(1, QB // TC)
                                j0 = q0 // TC
                                c0 = q0 % TC
                                if nj == 1:
                                    nc.sync.dma_start(
                                        a2a_in[hl][j0, :, c0:c0 + QB], ot_sb[:])
                                else:
                                    nc.sync.dma_start(
                                        a2a_in[hl][j0:j0 + nj, :, :],
                                        ot_sb.rearrange("p (j c) -> j p c",
                                                        j=nj))
                        nc.gpsimd.collective_compute(
                            "AllToAll", ALU.bypass,
                            replica_groups=[list(range(n_cores))],
                            ins=[a2a_in[hl][:]],
                            outs=[a2a_out[hl][:]])
                        # pull this head's reshard result into SBUF right away
                        # (gpsimd queue - keeps the sync queue free)
                        for i in range(n_cores):
                            g = i * QHC + hl
                            nc.gpsimd.dma_start(
                                ot_loc[:, g * TC:(g + 1) * TC],
                                a2a_out[hl][i, :, :])

                tc.no_sync_barrier()
                # ---------------- Phase 3: output projection ----------------
                # Each 1024-col wo block runs as two 512-col PSUM half-groups
                # (4 banks x 2 bufs) so drains overlap the next accumulation.
                with tc.tile_pool(name="ops", bufs=2, space="PSUM") as ops, \
                     tc.tile_pool(name="oout", bufs=4) as ooutp:

                    def prefetch_wo(mp):
                        # stream the next wo block on BOTH hwdge queues
                        # (sync + act) - one ring can't keep up with the
                        # PE's weight consumption rate
                        for gi2, g in enumerate(d_order):
                            wt = wotp.tile([128, 1024], CD, tag="wot",
                                           name=f"wot_{mp}_{g}")
                            wot_tiles[(mp, g)] = wt
                            eng = nc.sync if gi2 % 2 == 0 else nc.scalar
                            eng.dma_start(wt[:], woT[:, g, mp, :])

                    def mm(pos, sub, gi):
                        g = d_order[gi]
                        wt = wot_tiles[(mp, g)]
                        for tt in range(NT):
                            nc.tensor.matmul(
                                pos[tt],
                                ot_loc[:, g * TC + tt * 128:
                                       g * TC + (tt + 1) * 128],
                                wt[:, sub * 512:(sub + 1) * 512],
                                start=(gi == 0), stop=(gi == NH - 1))

                    def drain(pos, mp, sub):
                        # alternate PSUM drains between Act and the (idle in
                        # phase 3) DVE so pos banks free for the next group
                        # twice as fast
                        for tt in range(NT):
                            ob = ooutp.tile([128, 512], F32, tag="ob",
                                            name="ob")
                            if tt % 2 == 0:
                                nc.scalar.activation(ob[:], pos[tt], AF.Copy)
                            else:
                                nc.vector.tensor_copy(ob[:], pos[tt])
                            nc.sync.dma_start(
                                out_c[tt * 128:(tt + 1) * 128,
                                      (2 * mp + sub) * 512:
                                      (2 * mp + sub + 1) * 512],
                                ob[:])

                    for mp in range(NMB // 2):
                        if mp == 0:
                            # interleave both PSUM half-groups in gi-chunks:
                            # the first 24 g's (heads 0-2) give the PE ~190
                            # matmuls of runway that hides head 3's A2A
                            pos2 = [[ops.tile([128, 512], F32,
                                              tag=f"po{tt}",
                                              name=f"po_0_{sub}_{tt}")
                                     for tt in range(NT)]
                                    for sub in range(2)]
                            for sub in range(2):
                                for gi in range(0, 3 * NH // 4):
                                    mm(pos2[sub], sub, gi)
                            prefetch_wo(1)
                            for sub in range(2):
                                for gi in range(3 * NH // 4, NH):
                                    mm(pos2[sub], sub, gi)
                            for sub in range(2):
                                drain(pos2[sub], 0, sub)
                        else:
                            for sub in range(2):
                                pos = [ops.tile([128, 512], F32,
                                                tag=f"po{tt}",
                                                name=f"po_{mp}_{sub}_{tt}")
                                       for tt in range(NT)]
                                for gi in range(NH):
                                    mm(pos, sub, gi)
                                if sub == 0 and mp + 1 < NMB // 2:
                                    prefetch_wo(mp + 1)
                                drain(pos, mp, sub)

    nc.compile()
    return nc


def _np16(x):
    return np.asarray(x, dtype=ml_dtypes.bfloat16)


def _np8(x):
    return np.asarray(np.asarray(x, np.float32) * QS, dtype=ml_dtypes.float8_e4m3)


def prep_inputs(hidden_states, attention_mask, cos, sin, wq, wk, wv, wo,
                compute_dtype="bf16", n_cores=N_CORES):
    """Host-side sharding + transposes. Returns (in_maps, causal, dims)."""
    B, S, H = hidden_states.shape
    T = B * S
    D = NH * HD // n_cores
    KD = NKV * HD // n_cores
    cd = (lambda x: _np16(x)) if compute_dtype == "bf16" else \
         (lambda x: np.ascontiguousarray(x, dtype=np.float32))

    m = np.asarray(attention_mask, np.float32)[0, 0]
    expected = np.where(np.tril(np.ones((S, S), bool)), 0.0, NEG).astype(np.float32)
    causal = bool(np.array_equal(m, expected))
    use_fp8 = compute_dtype == "bf16" and causal

    hs2 = np.asarray(hidden_states, np.float32).reshape(T, H)
    # pre-tile hidden states: [128, NIT*2, nh2*TI] with one contiguous row
    # per partition per SBUF tile (TI=512 tokens, nh2=16 h-tiles per half)
    TI, nh2 = 512, H // 256
    NIT = T // TI
    hs4 = hs2.T.reshape(2, nh2, 128, NIT, TI)
    hsR = np.ascontiguousarray(hs4.transpose(2, 3, 0, 1, 4)).reshape(
        128, NIT * 2, nh2 * TI)

    def wtile(w, cols):
        # [cols*HT_rows, ...] -> [128, HT*cols]: tile ht on free dim
        wT = np.asarray(w, np.float32).T      # [H, cols]
        ht = H // 128
        return np.ascontiguousarray(
            wT.reshape(ht, 128, cols).transpose(1, 0, 2)).reshape(128, ht * cols)

    # wo: [128, NH, NMB/2, 1024] with p = row within 128-block g
    wo4 = np.asarray(wo, np.float32).T.reshape(NH, 128, H // 1024, 1024)
    woR = np.ascontiguousarray(wo4.transpose(1, 0, 2, 3))
    woT = cd(woR)
    cosT = np.ascontiguousarray(np.asarray(cos, np.float32)[0].T)
    sinT = np.ascontiguousarray(np.asarray(sin, np.float32)[0].T)
    sgnT = np.concatenate([-sinT[0:HD // 2], sinT[HD // 2:]], axis=0)
    sgnT = np.ascontiguousarray(sgnT)
    hsRc = cd(hsR)
    if use_fp8:
        hs8R = _np8(hsR)

    TC = T // n_cores
    QB = min(512, TC)
    KB = QB // 128
    in_maps = []
    for c in range(n_cores):
        im = {
            "hsR": hsRc,
            "wvT": cd(wtile(np.asarray(wv, np.float32)[c * KD:(c + 1) * KD],
                            KD)),
            "woT": woT,
            "cosT": cosT,
            "sgnT": sgnT,
        }
        wqs = np.asarray(wq, np.float32)[c * D:(c + 1) * D]
        wks = np.asarray(wk, np.float32)[c * KD:(c + 1) * KD]
        if use_fp8:
            im["hs8R"] = hs8R
            im["wq8T"] = np.asarray(wtile(wqs, D) * QS,
                                    dtype=ml_dtypes.float8_e4m3)
            im["wk8T"] = np.asarray(wtile(wks, KD) * QS,
                                    dtype=ml_dtypes.float8_e4m3)
        else:
            im["wqT"] = cd(wtile(wqs, D))
            im["wkT"] = cd(wtile(wks, KD))
        if causal:
            pk = np.arange(128)[:, None]
            pq = np.arange(QB)[None, :]
            dmask = np.concatenate(
                [np.where(pk + j * 128 <= pq, 1.0, 0.0) for j in range(KB)],
                axis=1).astype(np.float32)
            im["dmask"] = cd(dmask)
        else:
            im["maskT"] = np.ascontiguousarray(m.T)
        in_maps.append(im)
    return in_maps, causal, (T, S, H)


_BUILD_CACHE = {}


def kernel(hidden_states, attention_mask, cos, sin, wq, wk, wv, wo,
           compute_dtype="bf16", trace=False):
    B, S, H = hidden_states.shape
    T = B * S
    in_maps, causal, dims = prep_inputs(
        hidden_states, attention_mask, cos, sin, wq, wk, wv, wo,
        compute_dtype=compute_dtype)
    key = (T, S, H, compute_dtype, causal)
    if key not in _BUILD_CACHE:
        _BUILD_CACHE[key] = build(T, S, H, compute_dtype=compute_dtype,
                                  causal=causal)
    nc = _BUILD_CACHE[key]
    res = run_bass_kernel_spmd(nc, in_maps, core_ids=list(range(N_CORES)),
                               trace=trace)
    TC = T // N_CORES
    out = np.empty((T, H), np.float32)
    for c in range(N_CORES):
        out[c * TC:(c + 1) * TC] = res.results[c]["out"]
    if trace:
        kernel.last_exec_time_ns = res.exec_time_ns
        kernel.last_results = res
    return out.reshape(B, S, H)
